# revision 24
# baseline (speedup 1.0000x reference)
"""Bass/Trainium2 kernel for nn_Attention (B=2, N=2048, C=768, H=12).

Sharding (per the tensor-parallel-on-H hint): 8 cores = 2 batches x 4
head-triples. Core (b, hh) computes Q/K/V projections for heads
{3hh, 3hh+1, 3hh+2} over the FULL 2048-token sequence of batch b, the
attention for those heads, and the partial output projection
y_partial = (attn_out * gate) @ w_proj[rows of those heads]. The host-side
unshard sums the 4 partial y's per batch (row-parallel w_proj => the
output reduction is the unshard) and adds b_proj. No K/V duplication, no
collectives, ~40% less PE work per core than the query-sharded layout.

Attention math matches the baseline kernel: scores computed transposed
(S^T[key, query]) in 512-query blocks, softmax skips max-subtraction
(scores bounded for this distribution), denominator via a ones-column
appended to each head's V, exp split between ScalarE (ACT Exp) and
VectorE (Schraudolph bf16 bit-trick: int16(x*128/ln2 + (16256-5.5))
bitcast to bf16), softmax scale folded into W_q, per-head gate folded
into W_proj rows.

Schedule notes (HAM keeps the PE at 1.2 GHz unless its activity window
stays dense, so the emission order is arranged to avoid PE idle):
- x^T arrives as four 512-key chunks; per chunk the K, V and Q
  projections run back-to-back so compute starts ~3 us in and never
  waits for the tail of the x DMA.
- Q^T/K^T live in per-chunk tiles so the first score matmuls depend
  only on the first chunk's copies, not the last.
- Heads A,B run as concurrent K=64 row-group matmuls (A chans in
  partitions 0-63, B in 64-127). Head C's Q^T/K^T are stored duplicated
  in both partition halves (free: the projection runs two concurrent
  column-group matmuls with the same weights) so its score matmuls
  process two key-tiles per slot the same way.
- Each query block runs phase AB then phase C; score tiles are
  [128,512] (1 PSUM bank) so the st pool (bufs=5) holds >1 group of
  lookahead; A/B softmax-normalize overlaps phase C.
- The previous block's partial-y matmuls are woven into both phases
  (g>=2, after its normalize has settled) so the PE stays fed across
  block boundaries; the last block's y-projection is the tail.
- Output returns as bf16; host upcasts, sums partials, adds bias.
"""

import numpy as np
import ml_dtypes

B, N, C = 2, 2048, 768
H = 12
DH = C // H
SCALE = DH**-0.5
P = 128
HL = 3  # heads per core
KJ = C // P  # 6 contraction tiles over C
KT = N // P  # 16 key tiles
NB = N // 512  # 4 query blocks / x chunks
CW = HL * DH  # 192 qk channels per core
VW = HL * (DH + 1)  # 195 v columns (ones col per head)

EXP_C1 = 128.0 / float(np.log(2.0))
EXP_C2 = 16256.0 - 5.5

NCORES = 8
TRACE = False  # test.py flips this to profile
LAST_RESULT = None

_BF16 = ml_dtypes.bfloat16

_nc_cache = None


def _build_nc():
    from contextlib import ExitStack

    import concourse.tile as tile
    from concourse import bacc, mybir

    dt = mybir.dt
    F32, BF16, I16 = dt.float32, dt.bfloat16, dt.int16
    AF = mybir.ActivationFunctionType
    ALU = mybir.AluOpType

    nc = bacc.Bacc("TRN2", target_bir_lowering=False, num_devices=NCORES)

    xt = [
        nc.dram_tensor(f"xt{n}", [P, KJ * 512], BF16, kind="ExternalInput")
        for n in range(NB)
    ]
    wq = nc.dram_tensor("wq", [P, KJ * CW], BF16, kind="ExternalInput")
    wk = nc.dram_tensor("wk", [P, KJ * CW], BF16, kind="ExternalInput")
    wv = nc.dram_tensor("wv", [P, KJ * VW], BF16, kind="ExternalInput")
    wpp = nc.dram_tensor("wpp", [P, C], BF16, kind="ExternalInput")  # pair rows
    wpc = nc.dram_tensor("wpc", [64, C], BF16, kind="ExternalInput")  # head C rows
    out = nc.dram_tensor("out", [N, C], BF16, kind="ExternalOutput")

    with tile.TileContext(nc) as tc, ExitStack() as ctx:
        ps_pool = ctx.enter_context(tc.tile_pool(name="persist", bufs=1))

        xT = [
            ps_pool.tile([P, KJ, 512], BF16, tag=f"xT{n}", name=f"xT{n}")
            for n in range(NB)
        ]
        wq_s = ps_pool.tile([P, KJ * CW], BF16, tag="wq")
        wk_s = ps_pool.tile([P, KJ * CW], BF16, tag="wk")
        wv_s = ps_pool.tile([P, KJ * VW], BF16, tag="wv")
        wpp_s = ps_pool.tile([P, C], BF16, tag="wpp")
        wpc_s = ps_pool.tile([64, C], BF16, tag="wpc")
        # per-chunk projections: A chans 0-63 / B 64-127; C duplicated halves
        qTp = [ps_pool.tile([P, 512], BF16, tag=f"qTp{n}", name=f"qTp{n}") for n in range(NB)]
        kTp = [ps_pool.tile([P, 512], BF16, tag=f"kTp{n}", name=f"kTp{n}") for n in range(NB)]
        qTc = [ps_pool.tile([P, 512], BF16, tag=f"qTc{n}", name=f"qTc{n}") for n in range(NB)]
        kTc = [ps_pool.tile([P, 512], BF16, tag=f"kTc{n}", name=f"kTc{n}") for n in range(NB)]
        vsb = [ps_pool.tile([P, VW], BF16, tag=f"v{t}", name=f"v{t}") for t in range(KT)]
        otP = [
            ps_pool.tile([P, 512], BF16, tag=f"otP{q}", name=f"otP{q}") for q in range(NB)
        ]
        otC = [
            ps_pool.tile([64, 512], BF16, tag=f"otC{q}", name=f"otC{q}") for q in range(NB)
        ]

        def kslice(kTx, kt):
            # key tile kt inside the per-chunk K^T tiles
            return kTx[kt // 4][:, (kt % 4) * P : (kt % 4 + 1) * P]

        # ---- input loads (one HWDGE ring, FIFO) ----
        nc.sync.dma_start(wk_s[:], wk[:])
        nc.sync.dma_start(xT[0][:], xt[0][:].rearrange("p (j n) -> p j n", n=512))
        nc.sync.dma_start(wv_s[:], wv[:])
        nc.sync.dma_start(wq_s[:], wq[:])
        nc.sync.dma_start(xT[1][:], xt[1][:].rearrange("p (j n) -> p j n", n=512))
        nc.sync.dma_start(xT[2][:], xt[2][:].rearrange("p (j n) -> p j n", n=512))
        nc.sync.dma_start(xT[3][:], xt[3][:].rearrange("p (j n) -> p j n", n=512))
        nc.sync.dma_start(wpp_s[:], wpp[:])
        nc.sync.dma_start(wpc_s[:], wpc[:])

        with (
            tc.tile_pool(name="st", bufs=5, space="PSUM") as stp,
            tc.tile_pool(name="ot", bufs=3, space="PSUM") as otp,
            tc.tile_pool(name="pexp", bufs=18) as pexp,
        ):
            def proj_pair(w_s, dst, nt):
                ps = stp.tile([P, 512], F32, tag="st", name=f"pp{dst.name}")
                for j in range(KJ):
                    nc.tensor.matmul(
                        ps[:],
                        lhsT=w_s[:, j * CW : j * CW + P],
                        rhs=xT[nt][:, j, :],
                        start=(j == 0),
                        stop=(j == KJ - 1),
                    )
                nc.vector.tensor_copy(dst[:], ps[:])

            def proj_c(nt):
                # head C: Q and K projections share the same rhs stream via
                # two concurrent column groups (Q -> partitions 0-63,
                # K -> 64-127); the partition-half duplicates the score
                # matmuls need are made by SBUF->SBUF DMAs on the idle ring
                ps = stp.tile([P, 512], F32, tag="st", name=f"pqk{nt}")
                for j in range(KJ):
                    nc.tensor.matmul(
                        ps[0:64, :],
                        lhsT=wq_s[:, j * CW + 2 * DH : j * CW + CW],
                        rhs=xT[nt][:, j, :],
                        start=(j == 0),
                        stop=(j == KJ - 1),
                        tile_position=(0, 0),
                    )
                    nc.tensor.matmul(
                        ps[64:128, :],
                        lhsT=wk_s[:, j * CW + 2 * DH : j * CW + CW],
                        rhs=xT[nt][:, j, :],
                        start=(j == 0),
                        stop=(j == KJ - 1),
                        tile_position=(0, 64),
                    )
                nc.vector.tensor_copy(qTc[nt][0:64, :], ps[0:64, :])
                nc.vector.tensor_copy(kTc[nt][64:128, :], ps[64:128, :])
                nc.scalar.dma_start(qTc[nt][64:128, :], qTc[nt][0:64, :])
                nc.scalar.dma_start(kTc[nt][0:64, :], kTc[nt][64:128, :])

            def proj_v(t):
                ps = stp.tile([P, 512], F32, tag="st", name=f"psv{t}")
                for j in range(KJ):
                    nc.tensor.matmul(
                        ps[:, 0:VW],
                        lhsT=xT[t // 4][:, j, (t % 4) * P : (t % 4 + 1) * P],
                        rhs=wv_s[:, j * VW : (j + 1) * VW],
                        start=(j == 0),
                        stop=(j == KJ - 1),
                    )
                nc.scalar.copy(vsb[t][:], ps[:, 0:VW])
                ones_ap = vsb[t][:].rearrange("p (h d) -> p h d", d=DH + 1)[:, :, DH : DH + 1]
                nc.vector.memset(ones_ap, 1.0)

            # per x-chunk: K, V, Q back-to-back (starts as soon as the
            # chunk and the respective weights land)
            # HAM warm-up: DMA-independent matmuls on a memset tile keep
            # the PE activity window busy through the ~9 us queue bring-up
            # before the first input byte lands, so the projections run at
            # 2.4 GHz from their first instruction
            wsrc = pexp.tile([P, 512], BF16, tag="wsrc", bufs=1, name="wsrc")
            nc.vector.memset(wsrc[:], 0.0)
            warm = stp.tile([P, 512], F32, tag="st", name="warm")
            for i in range(26):
                nc.tensor.matmul(
                    warm[:], lhsT=wsrc[:, 0:P], rhs=wsrc[:], start=True, stop=True,
                )
            wdump = pexp.tile([P, 4], F32, tag="rc", bufs=6, name="wdump")
            nc.scalar.copy(wdump[:], warm[:, 0:4])

            for nt in range(NB):
                proj_pair(wk_s, kTp[nt], nt)
                proj_c(nt)
                for lt in range(4):
                    proj_v(4 * nt + lt)
                proj_pair(wq_s, qTp[nt], nt)

            def exp_act(dst, src):
                nc.scalar.activation(dst[:], src[:], AF.Exp)

            def exp_dve(dst, src):
                nc.vector.tensor_scalar(
                    dst[:].bitcast(I16), src[:], EXP_C1, EXP_C2,
                    op0=ALU.mult, op1=ALU.add,
                )

            ysb_tiles = {}

            def y_unit(qb, qt, piece):
                # one quarter-tile, half-width piece of the partial
                # y-projection for block qb; woven into the next block's
                # score loops to keep the PE fed across block boundaries
                lp = otP[qb][:, qt * P : (qt + 1) * P]
                lc = otC[qb][:, qt * P : (qt + 1) * P]
                if piece == 0:
                    ysb_tiles[(qb, qt)] = pexp.tile(
                        [P, C], BF16, tag="y", bufs=4, name=f"ysb{qb}_{qt}"
                    )
                ysb = ysb_tiles[(qb, qt)]
                o0 = piece * 384
                ps = stp.tile([P, 512], F32, tag="st", name=f"psy{qb}_{qt}_{piece}")
                nc.tensor.matmul(
                    ps[:, 0:384],
                    lhsT=lp,
                    rhs=wpp_s[:, o0 : o0 + 384],
                    start=True,
                    stop=False,
                )
                nc.tensor.matmul(
                    ps[:, 0:384],
                    lhsT=lc,
                    rhs=wpc_s[0:64, o0 : o0 + 384],
                    start=False,
                    stop=True,
                )
                if piece == 0:
                    nc.scalar.copy(ysb[:, o0 : o0 + 384], ps[:, 0:384])
                else:
                    nc.vector.tensor_copy(ysb[:, o0 : o0 + 384], ps[:, 0:384])
                if piece == 1:
                    nc.sync.dma_start(
                        out[qb * 512 + qt * P : qb * 512 + (qt + 1) * P, :], ysb[:]
                    )
                    del ysb_tiles[(qb, qt)]

            def normalize(ot, dst_ap, tag):
                # softmax denominator: ones row = partition 64 of ot
                rc = pexp.tile([1, 512], F32, tag="rc", bufs=6, name=f"rc{tag}")
                sg = pexp.tile([1, 512], F32, tag="sg", bufs=6, name=f"sg{tag}")
                nc.vector.tensor_copy(sg[:], ot[64:65, :])
                nc.vector.reciprocal_approx_fast(rc[:], sg[:])
                rb = pexp.tile([64, 512], F32, tag="rb", bufs=6, name=f"rb{tag}")
                nc.gpsimd.partition_broadcast(rb[:], rc[:])
                nc.vector.tensor_mul(dst_ap, ot[0:64, :], rb[:])

            def attention(qb, ys, c_first=False):
                q0, q1 = qb * 512, (qb + 1) * 512
                if c_first:
                    attention_c(qb, ys)
                # ---- phase AB ----
                otA = otp.tile([DH + 1, 512], F32, tag="ot", name=f"otA{qb}")
                otB = otp.tile([DH + 1, 512], F32, tag="ot", name=f"otB{qb}")
                hist = []
                for g in range(KT // 2 + 2):
                    if g < KT // 2:
                        sts = [
                            stp.tile([P, 512], F32, tag="st", name=f"sab{qb}_{g}_{x}")
                            for x in range(4)
                        ]  # A0 B0 A1 B1
                        for u in range(2):
                            kt = 2 * g + u
                            nc.tensor.matmul(
                                sts[2 * u][:],
                                lhsT=kslice(kTp, kt)[0:64, :],
                                rhs=qTp[qb][0:64, :],
                                start=True, stop=True,
                                tile_position=(0, 0),
                            )
                            nc.tensor.matmul(
                                sts[2 * u + 1][:],
                                lhsT=kslice(kTp, kt)[64:128, :],
                                rhs=qTp[qb][64:128, :],
                                start=True, stop=True,
                                tile_position=(64, 0),
                            )
                        ps4 = [
                            pexp.tile([P, 512], BF16, tag="pexp", name=f"pab{qb}_{g}_{x}")
                            for x in range(4)
                        ]
                        exp_act(ps4[0], sts[0])
                        exp_dve(ps4[1], sts[1])
                        exp_act(ps4[2], sts[2])
                        exp_dve(ps4[3], sts[3])
                        hist.append((g, ps4))
                    if g >= 2 and hist:
                        pg, pp4 = hist.pop(0)
                        for _ in range(2):
                            if ys:
                                yu = ys.pop()
                                if yu is not None:
                                    y_unit(*yu)
                        for u in range(2):
                            kt = 2 * pg + u
                            nc.tensor.matmul(
                                otA[:],
                                lhsT=vsb[kt][:, 0 : DH + 1],
                                rhs=pp4[2 * u][:],
                                start=(kt == 0), stop=(kt == KT - 1),
                            )
                            nc.tensor.matmul(
                                otB[:],
                                lhsT=vsb[kt][:, DH + 1 : 2 * (DH + 1)],
                                rhs=pp4[2 * u + 1][:],
                                start=(kt == 0), stop=(kt == KT - 1),
                            )
                normalize(otA, otP[qb][0:64, :], f"A{qb}")
                normalize(otB, otP[qb][64:128, :], f"B{qb}")
                if not c_first:
                    attention_c(qb, ys)

            def attention_c(qb, ys):
                q0, q1 = qb * 512, (qb + 1) * 512
                otCc = otp.tile([DH + 1, 512], F32, tag="ot", name=f"otC{qb}")
                hist = []
                for g in range(KT // 2 + 2):
                    if g < KT // 2:
                        stC = [
                            stp.tile([P, 512], F32, tag="st", name=f"sc{qb}_{g}_{x}")
                            for x in range(2)
                        ]
                        nc.tensor.matmul(
                            stC[0][:],
                            lhsT=kslice(kTc, 2 * g)[0:64, :],
                            rhs=qTc[qb][0:64, :],
                            start=True, stop=True,
                            tile_position=(0, 0),
                        )
                        nc.tensor.matmul(
                            stC[1][:],
                            lhsT=kslice(kTc, 2 * g + 1)[64:128, :],
                            rhs=qTc[qb][64:128, :],
                            start=True, stop=True,
                            tile_position=(64, 0),
                        )
                        pc = [
                            pexp.tile([P, 512], BF16, tag="pexp", name=f"pc{qb}_{g}_{x}")
                            for x in range(2)
                        ]
                        # ACT is the in-block co-bottleneck: hand every 4th
                        # group's C exps to the DVE to level the queues
                        exp_c = exp_act if g % 4 != 3 else exp_dve
                        exp_c(pc[0], stC[0])
                        exp_c(pc[1], stC[1])
                        hist.append((g, pc))
                    if g >= 2 and hist:
                        pg, ppc = hist.pop(0)
                        for _ in range(2):
                            if ys:
                                yu = ys.pop()
                                if yu is not None:
                                    y_unit(*yu)
                        for u in range(2):
                            kt = 2 * pg + u
                            nc.tensor.matmul(
                                otCc[:],
                                lhsT=vsb[kt][:, 2 * (DH + 1) : VW],
                                rhs=ppc[u][:],
                                start=(kt == 0), stop=(kt == KT - 1),
                            )
                normalize(otCc, otC[qb][:], f"C{qb}")

            ymap = {2: [0], 3: [1, 2]}
            for qb in range(NB):
                units = [
                    (src_qb, qt, pc)
                    for src_qb in ymap.get(qb, [])
                    for qt in range(4)
                    for pc in range(2)
                ]
                attention(qb, list(reversed(units)), c_first=(qb == NB - 1))

            # tail: block 3's y. Head-C matmuls first (otC[3] is ready --
            # phase C ran first), overlapping the A/B normalize; pair
            # matmuls follow in separate PSUM tiles, merged by the add.
            lastq = NB - 1
            cps = {}
            pend = []

            def tail_c(qt, piece):
                lc = otC[lastq][:, qt * P : (qt + 1) * P]
                o0 = piece * 384
                ps = otp.tile([P, 512], F32, tag="ot", name=f"tc{qt}_{piece}")
                nc.tensor.matmul(
                    ps[:, 0:384], lhsT=lc, rhs=wpc_s[0:64, o0 : o0 + 384],
                    start=True, stop=True,
                )
                csb = pexp.tile([P, 384], F32, tag="csb", bufs=4, name=f"cs{qt}_{piece}")
                nc.scalar.copy(csb[:], ps[:, 0:384])
                cps[(qt, piece)] = csb

            def tail_p(qt, piece):
                lp = otP[lastq][:, qt * P : (qt + 1) * P]
                if piece == 0:
                    ysb_tiles[(lastq, qt)] = pexp.tile(
                        [P, C], BF16, tag="y", bufs=4, name=f"ysbL{qt}"
                    )
                ysb = ysb_tiles[(lastq, qt)]
                o0 = piece * 384
                ps = stp.tile([P, 512], F32, tag="st", name=f"tp{qt}_{piece}")
                nc.tensor.matmul(
                    ps[:, 0:384], lhsT=lp, rhs=wpp_s[:, o0 : o0 + 384],
                    start=True, stop=True,
                )
                csb = cps.pop((qt, piece))
                nc.vector.tensor_add(ysb[:, o0 : o0 + 384], ps[:, 0:384], csb[:])
                if piece == 1:
                    nc.sync.dma_start(
                        out[lastq * 512 + qt * P : lastq * 512 + (qt + 1) * P, :],
                        ysb[:],
                    )

            order = [(qt, pc) for qt in range(4) for pc in range(2)]
            # ladder: C-matmuls run ~3 units ahead of pair-matmuls
            emitted_c = 0
            for i, (qt, pc) in enumerate(order):
                while emitted_c < min(len(order), i + 3):
                    tail_c(*order[emitted_c])
                    emitted_c += 1
                tail_p(qt, pc)

    nc.compile()
    return nc


def _get_nc():
    global _nc_cache
    if _nc_cache is None:
        _nc_cache = _build_nc()
    return _nc_cache


def _ktile_major(w):
    # [C, M] -> [128, KJ*M] with contraction tile j at free offset j*M
    M = w.shape[1]
    return np.ascontiguousarray(
        w.reshape(KJ, P, M).transpose(1, 0, 2).reshape(P, KJ * M)
    )


def kernel(x, w_qkv, gate, w_proj, b_proj):
    from concourse import bass_utils

    global LAST_RESULT

    x = np.asarray(x, dtype=np.float32)
    w_qkv = np.asarray(w_qkv, dtype=np.float32)
    gate = np.asarray(gate, dtype=np.float32)
    w_proj = np.asarray(w_proj, dtype=np.float32)
    b_proj = np.asarray(b_proj, dtype=np.float32)

    # ---- host-side layout prep (weights folded/sliced, layout-only for x) ----
    wq_full = w_qkv[:, 0:C] * SCALE
    wk_full = w_qkv[:, C : 2 * C]
    wv_full = w_qkv[:, 2 * C : 3 * C]
    gated_wp = w_proj * np.repeat(gate, DH)[:, None]

    per_hh = []
    for hh in range(4):
        h0 = HL * hh
        cs = slice(h0 * DH, (h0 + HL) * DH)
        wq_np = _ktile_major(wq_full[:, cs]).astype(_BF16)
        wk_np = _ktile_major(wk_full[:, cs]).astype(_BF16)
        wv_pad = np.zeros((C, VW), dtype=np.float32)
        for h in range(HL):
            wv_pad[:, h * (DH + 1) : h * (DH + 1) + DH] = wv_full[
                :, (h0 + h) * DH : (h0 + h + 1) * DH
            ]
        wv_np = _ktile_major(wv_pad).astype(_BF16)
        wp_rows = gated_wp[cs, :]  # [192, 768]
        wpp_np = np.ascontiguousarray(wp_rows[0 : 2 * DH, :]).astype(_BF16)
        wpc_np = np.ascontiguousarray(wp_rows[2 * DH :, :]).astype(_BF16)
        per_hh.append((wq_np, wk_np, wv_np, wpp_np, wpc_np))

    xt_b = []
    for b in range(B):
        xtb = x[b].T.astype(_BF16)  # [C, N]
        xt_b.append(
            [
                _ktile_major(np.ascontiguousarray(xtb[:, n * 512 : (n + 1) * 512]))
                for n in range(NB)
            ]
        )

    in_maps = []
    for c in range(NCORES):
        b, hh = c // 4, c % 4
        wq_np, wk_np, wv_np, wpp_np, wpc_np = per_hh[hh]
        m = {f"xt{n}": xt_b[b][n] for n in range(NB)}
        m.update({"wq": wq_np, "wk": wk_np, "wv": wv_np, "wpp": wpp_np, "wpc": wpc_np})
        in_maps.append(m)

    nc = _get_nc()
    # the first execution of a freshly compiled NEFF occasionally hits a
    # transient NRT_EXEC_UNIT_UNRECOVERABLE; a retry reliably succeeds
    last_exc = None
    for _attempt in range(3):
        try:
            res = bass_utils.run_bass_kernel_spmd(
                nc, in_maps, core_ids=list(range(NCORES)), trace=TRACE
            )
            break
        except Exception as e:  # noqa: BLE001
            last_exc = e
    else:
        raise last_exc
    LAST_RESULT = res

    out = np.empty((B, N, C), dtype=np.float32)
    for b in range(B):
        acc = np.asarray(res.results[4 * b + 0]["out"]).astype(np.float32)
        for hh in range(1, 4):
            acc += np.asarray(res.results[4 * b + hh]["out"]).astype(np.float32)
        out[b] = acc + b_proj[None, :]
    return out


# revision 25
# speedup vs baseline: 1.0234x; 1.0234x over previous
"""Bass/Trainium2 kernel for nn_Attention (B=2, N=2048, C=768, H=12).

Sharding (per the tensor-parallel-on-H hint): 8 cores = 2 batches x 4
head-triples. Core (b, hh) computes Q/K/V projections for heads
{3hh, 3hh+1, 3hh+2} over the FULL 2048-token sequence of batch b, the
attention for those heads, and the partial output projection
y_partial = (attn_out * gate) @ w_proj[rows of those heads]. The host-side
unshard sums the 4 partial y's per batch (row-parallel w_proj => the
output reduction is the unshard) and adds b_proj. No K/V duplication, no
collectives, ~40% less PE work per core than the query-sharded layout.

Attention math matches the baseline kernel: scores computed transposed
(S^T[key, query]) in 512-query blocks, softmax skips max-subtraction
(scores bounded for this distribution), denominator via a ones-column
appended to each head's V, exp split between ScalarE (ACT Exp) and
VectorE (Schraudolph bf16 bit-trick: int16(x*128/ln2 + (16256-5.5))
bitcast to bf16), softmax scale folded into W_q, per-head gate folded
into W_proj rows.

Schedule notes (HAM keeps the PE at 1.2 GHz unless its activity window
stays dense, so the emission order is arranged to avoid PE idle):
- x^T arrives as four 512-key chunks; per chunk the K, V and Q
  projections run back-to-back so compute starts ~3 us in and never
  waits for the tail of the x DMA.
- Q^T/K^T live in per-chunk tiles so the first score matmuls depend
  only on the first chunk's copies, not the last.
- Heads A,B run as concurrent K=64 row-group matmuls (A chans in
  partitions 0-63, B in 64-127). Head C's Q^T/K^T are stored duplicated
  in both partition halves (free: the projection runs two concurrent
  column-group matmuls with the same weights) so its score matmuls
  process two key-tiles per slot the same way.
- Each query block runs phase AB then phase C; score tiles are
  [128,512] (1 PSUM bank) so the st pool (bufs=5) holds >1 group of
  lookahead; A/B softmax-normalize overlaps phase C.
- The previous block's partial-y matmuls are woven into both phases
  (g>=2, after its normalize has settled) so the PE stays fed across
  block boundaries; the last block's y-projection is the tail.
- Output returns as bf16; host upcasts, sums partials, adds bias.
"""

import numpy as np
import ml_dtypes

B, N, C = 2, 2048, 768
H = 12
DH = C // H
SCALE = DH**-0.5
P = 128
HL = 3  # heads per core
KJ = C // P  # 6 contraction tiles over C
KT = N // P  # 16 key tiles
NB = N // 512  # 4 query blocks / x chunks
CW = HL * DH  # 192 qk channels per core
VW = HL * (DH + 1)  # 195 v columns (ones col per head)

EXP_C1 = 128.0 / float(np.log(2.0))
EXP_C2 = 16256.0 - 5.5

NCORES = 8
TRACE = False  # test.py flips this to profile
LAST_RESULT = None

_BF16 = ml_dtypes.bfloat16

_nc_cache = None


def _build_nc():
    from contextlib import ExitStack

    import concourse.tile as tile
    from concourse import bacc, mybir

    dt = mybir.dt
    F32, BF16, I16 = dt.float32, dt.bfloat16, dt.int16
    AF = mybir.ActivationFunctionType
    ALU = mybir.AluOpType

    nc = bacc.Bacc("TRN2", target_bir_lowering=False, num_devices=NCORES)

    xt = [
        nc.dram_tensor(f"xt{n}", [P, KJ * 512], BF16, kind="ExternalInput")
        for n in range(NB)
    ]
    wq = nc.dram_tensor("wq", [P, KJ * CW], BF16, kind="ExternalInput")
    wk = nc.dram_tensor("wk", [P, KJ * CW], BF16, kind="ExternalInput")
    wv = nc.dram_tensor("wv", [P, KJ * VW], BF16, kind="ExternalInput")
    wpp = nc.dram_tensor("wpp", [P, C], BF16, kind="ExternalInput")  # pair rows
    wpc = nc.dram_tensor("wpc", [64, C], BF16, kind="ExternalInput")  # head C rows
    out = nc.dram_tensor("out", [N, C], BF16, kind="ExternalOutput")

    with tile.TileContext(nc) as tc, ExitStack() as ctx:
        ps_pool = ctx.enter_context(tc.tile_pool(name="persist", bufs=1))

        xT = [
            ps_pool.tile([P, KJ, 512], BF16, tag=f"xT{n}", name=f"xT{n}")
            for n in range(NB)
        ]
        wq_s = ps_pool.tile([P, KJ * CW], BF16, tag="wq")
        wk_s = ps_pool.tile([P, KJ * CW], BF16, tag="wk")
        wv_s = ps_pool.tile([P, KJ * VW], BF16, tag="wv")
        wpp_s = ps_pool.tile([P, C], BF16, tag="wpp")
        wpc_s = ps_pool.tile([64, C], BF16, tag="wpc")
        # per-chunk projections: A chans 0-63 / B 64-127; C duplicated halves
        qTp = [ps_pool.tile([P, 512], BF16, tag=f"qTp{n}", name=f"qTp{n}") for n in range(NB)]
        kTp = [ps_pool.tile([P, 512], BF16, tag=f"kTp{n}", name=f"kTp{n}") for n in range(NB)]
        qTc = [ps_pool.tile([P, 512], BF16, tag=f"qTc{n}", name=f"qTc{n}") for n in range(NB)]
        kTc = [ps_pool.tile([P, 512], BF16, tag=f"kTc{n}", name=f"kTc{n}") for n in range(NB)]
        vsb = [ps_pool.tile([P, VW], BF16, tag=f"v{t}", name=f"v{t}") for t in range(KT)]
        otP = [
            ps_pool.tile([P, 512], BF16, tag=f"otP{q}", name=f"otP{q}") for q in range(NB)
        ]
        otC = [
            ps_pool.tile([64, 512], BF16, tag=f"otC{q}", name=f"otC{q}") for q in range(NB)
        ]

        def kslice(kTx, kt):
            # key tile kt inside the per-chunk K^T tiles
            return kTx[kt // 4][:, (kt % 4) * P : (kt % 4 + 1) * P]

        # ---- input loads (one HWDGE ring, FIFO) ----
        nc.sync.dma_start(wk_s[:], wk[:])
        nc.sync.dma_start(xT[0][:], xt[0][:].rearrange("p (j n) -> p j n", n=512))
        nc.sync.dma_start(wv_s[:], wv[:])
        nc.sync.dma_start(wq_s[:], wq[:])
        nc.sync.dma_start(xT[1][:], xt[1][:].rearrange("p (j n) -> p j n", n=512))
        nc.sync.dma_start(xT[2][:], xt[2][:].rearrange("p (j n) -> p j n", n=512))
        nc.sync.dma_start(xT[3][:], xt[3][:].rearrange("p (j n) -> p j n", n=512))
        nc.sync.dma_start(wpp_s[:], wpp[:])
        nc.sync.dma_start(wpc_s[:], wpc[:])

        with (
            tc.tile_pool(name="st", bufs=5, space="PSUM") as stp,
            tc.tile_pool(name="ot", bufs=3, space="PSUM") as otp,
            tc.tile_pool(name="pexp", bufs=12) as pexp,
        ):
            def proj_pair(w_s, dst, nt):
                ps = stp.tile([P, 512], F32, tag="st", name=f"pp{dst.name}")
                for j in range(KJ):
                    nc.tensor.matmul(
                        ps[:],
                        lhsT=w_s[:, j * CW : j * CW + P],
                        rhs=xT[nt][:, j, :],
                        start=(j == 0),
                        stop=(j == KJ - 1),
                    )
                nc.vector.tensor_copy(dst[:], ps[:])

            def proj_c(nt):
                # head C: Q and K projections share the same rhs stream via
                # two concurrent column groups (Q -> partitions 0-63,
                # K -> 64-127); the partition-half duplicates the score
                # matmuls need are made by SBUF->SBUF DMAs on the idle ring
                ps = stp.tile([P, 512], F32, tag="st", name=f"pqk{nt}")
                for j in range(KJ):
                    nc.tensor.matmul(
                        ps[0:64, :],
                        lhsT=wq_s[:, j * CW + 2 * DH : j * CW + CW],
                        rhs=xT[nt][:, j, :],
                        start=(j == 0),
                        stop=(j == KJ - 1),
                        tile_position=(0, 0),
                    )
                    nc.tensor.matmul(
                        ps[64:128, :],
                        lhsT=wk_s[:, j * CW + 2 * DH : j * CW + CW],
                        rhs=xT[nt][:, j, :],
                        start=(j == 0),
                        stop=(j == KJ - 1),
                        tile_position=(0, 64),
                    )
                nc.vector.tensor_copy(qTc[nt][0:64, :], ps[0:64, :])
                nc.vector.tensor_copy(kTc[nt][64:128, :], ps[64:128, :])
                nc.scalar.dma_start(qTc[nt][64:128, :], qTc[nt][0:64, :])
                nc.scalar.dma_start(kTc[nt][0:64, :], kTc[nt][64:128, :])

            def proj_v(t):
                ps = stp.tile([P, 512], F32, tag="st", name=f"psv{t}")
                for j in range(KJ):
                    nc.tensor.matmul(
                        ps[:, 0:VW],
                        lhsT=xT[t // 4][:, j, (t % 4) * P : (t % 4 + 1) * P],
                        rhs=wv_s[:, j * VW : (j + 1) * VW],
                        start=(j == 0),
                        stop=(j == KJ - 1),
                    )
                nc.scalar.copy(vsb[t][:], ps[:, 0:VW])
                ones_ap = vsb[t][:].rearrange("p (h d) -> p h d", d=DH + 1)[:, :, DH : DH + 1]
                nc.vector.memset(ones_ap, 1.0)

            # per x-chunk: K, V, Q back-to-back (starts as soon as the
            # chunk and the respective weights land)
            # HAM warm-up: DMA-independent matmuls on a memset tile keep
            # the PE activity window busy through the ~9 us queue bring-up
            # before the first input byte lands, so the projections run at
            # 2.4 GHz from their first instruction
            wsrc = pexp.tile([P, 512], BF16, tag="wsrc", bufs=1, name="wsrc")
            nc.vector.memset(wsrc[:], 0.0)
            warm = stp.tile([P, 512], F32, tag="st", name="warm")
            for i in range(26):
                nc.tensor.matmul(
                    warm[:], lhsT=wsrc[:, 0:P], rhs=wsrc[:], start=True, stop=True,
                )
            wdump = pexp.tile([P, 4], F32, tag="rc", bufs=6, name="wdump")
            nc.scalar.copy(wdump[:], warm[:, 0:4])

            for nt in range(NB):
                proj_pair(wk_s, kTp[nt], nt)
                proj_c(nt)
                for lt in range(4):
                    proj_v(4 * nt + lt)
                proj_pair(wq_s, qTp[nt], nt)

            def exp_act(dst, src):
                nc.scalar.activation(dst[:], src[:], AF.Exp)

            def exp_dve(dst, src):
                nc.vector.tensor_scalar(
                    dst[:].bitcast(I16), src[:], EXP_C1, EXP_C2,
                    op0=ALU.mult, op1=ALU.add,
                )

            ysb_tiles = {}

            def y_unit(qb, qt, piece):
                # one quarter-tile, half-width piece of the partial
                # y-projection for block qb; woven into the next block's
                # score loops to keep the PE fed across block boundaries
                lp = otP[qb][:, qt * P : (qt + 1) * P]
                lc = otC[qb][:, qt * P : (qt + 1) * P]
                if piece == 0:
                    ysb_tiles[(qb, qt)] = pexp.tile(
                        [P, C], BF16, tag="y", bufs=4, name=f"ysb{qb}_{qt}"
                    )
                ysb = ysb_tiles[(qb, qt)]
                o0 = piece * 384
                ps = stp.tile([P, 512], F32, tag="st", name=f"psy{qb}_{qt}_{piece}")
                nc.tensor.matmul(
                    ps[:, 0:384],
                    lhsT=lp,
                    rhs=wpp_s[:, o0 : o0 + 384],
                    start=True,
                    stop=False,
                )
                nc.tensor.matmul(
                    ps[:, 0:384],
                    lhsT=lc,
                    rhs=wpc_s[0:64, o0 : o0 + 384],
                    start=False,
                    stop=True,
                )
                if piece == 0:
                    nc.scalar.copy(ysb[:, o0 : o0 + 384], ps[:, 0:384])
                else:
                    nc.vector.tensor_copy(ysb[:, o0 : o0 + 384], ps[:, 0:384])
                if piece == 1:
                    nc.sync.dma_start(
                        out[qb * 512 + qt * P : qb * 512 + (qt + 1) * P, :], ysb[:]
                    )
                    del ysb_tiles[(qb, qt)]

            def normalize(ot, dst_ap, tag):
                # softmax denominator: ones row = partition 64 of ot
                rc = pexp.tile([1, 512], F32, tag="rc", bufs=6, name=f"rc{tag}")
                sg = pexp.tile([1, 512], F32, tag="sg", bufs=6, name=f"sg{tag}")
                nc.vector.tensor_copy(sg[:], ot[64:65, :])
                nc.vector.reciprocal_approx_fast(rc[:], sg[:])
                rb = pexp.tile([64, 512], F32, tag="rb", bufs=6, name=f"rb{tag}")
                nc.gpsimd.partition_broadcast(rb[:], rc[:])
                nc.vector.tensor_mul(dst_ap, ot[0:64, :], rb[:])

            def attention(qb, ys, c_first=False):
                q0, q1 = qb * 512, (qb + 1) * 512
                if c_first:
                    attention_c(qb, ys)
                # ---- phase AB ----
                otA = otp.tile([DH + 1, 512], F32, tag="ot", name=f"otA{qb}")
                otB = otp.tile([DH + 1, 512], F32, tag="ot", name=f"otB{qb}")
                prev = None
                for g in range(KT // 2 + 1):
                    if g < KT // 2:
                        sts = [
                            stp.tile([P, 512], F32, tag="st", name=f"sab{qb}_{g}_{x}")
                            for x in range(4)
                        ]  # A0 B0 A1 B1
                        for u in range(2):
                            kt = 2 * g + u
                            nc.tensor.matmul(
                                sts[2 * u][:],
                                lhsT=kslice(kTp, kt)[0:64, :],
                                rhs=qTp[qb][0:64, :],
                                start=True, stop=True,
                                tile_position=(0, 0),
                            )
                            nc.tensor.matmul(
                                sts[2 * u + 1][:],
                                lhsT=kslice(kTp, kt)[64:128, :],
                                rhs=qTp[qb][64:128, :],
                                start=True, stop=True,
                                tile_position=(64, 0),
                            )
                        ps4 = [
                            pexp.tile([P, 512], BF16, tag="pexp", name=f"pab{qb}_{g}_{x}")
                            for x in range(4)
                        ]
                        exp_act(ps4[0], sts[0])
                        exp_dve(ps4[1], sts[1])
                        exp_act(ps4[2], sts[2])
                        exp_dve(ps4[3], sts[3])
                    if prev is not None:
                        pg, pp4 = prev
                        for _ in range(2):
                            if ys:
                                yu = ys.pop()
                                if yu is not None:
                                    y_unit(*yu)
                        for u in range(2):
                            kt = 2 * pg + u
                            nc.tensor.matmul(
                                otA[:],
                                lhsT=vsb[kt][:, 0 : DH + 1],
                                rhs=pp4[2 * u][:],
                                start=(kt == 0), stop=(kt == KT - 1),
                            )
                            nc.tensor.matmul(
                                otB[:],
                                lhsT=vsb[kt][:, DH + 1 : 2 * (DH + 1)],
                                rhs=pp4[2 * u + 1][:],
                                start=(kt == 0), stop=(kt == KT - 1),
                            )
                    prev = (g, ps4) if g < KT // 2 else None
                normalize(otA, otP[qb][0:64, :], f"A{qb}")
                normalize(otB, otP[qb][64:128, :], f"B{qb}")
                if not c_first:
                    attention_c(qb, ys)

            def attention_c(qb, ys):
                q0, q1 = qb * 512, (qb + 1) * 512
                otCc = otp.tile([DH + 1, 512], F32, tag="ot", name=f"otC{qb}")
                prev = None
                for g in range(KT // 2 + 1):
                    if g < KT // 2:
                        stC = [
                            stp.tile([P, 512], F32, tag="st", name=f"sc{qb}_{g}_{x}")
                            for x in range(2)
                        ]
                        nc.tensor.matmul(
                            stC[0][:],
                            lhsT=kslice(kTc, 2 * g)[0:64, :],
                            rhs=qTc[qb][0:64, :],
                            start=True, stop=True,
                            tile_position=(0, 0),
                        )
                        nc.tensor.matmul(
                            stC[1][:],
                            lhsT=kslice(kTc, 2 * g + 1)[64:128, :],
                            rhs=qTc[qb][64:128, :],
                            start=True, stop=True,
                            tile_position=(64, 0),
                        )
                        pc = [
                            pexp.tile([P, 512], BF16, tag="pexp", name=f"pc{qb}_{g}_{x}")
                            for x in range(2)
                        ]
                        # ACT is the in-block co-bottleneck: hand every 4th
                        # group's C exps to the DVE to level the queues
                        exp_c = exp_act if g % 4 != 3 else exp_dve
                        exp_c(pc[0], stC[0])
                        exp_c(pc[1], stC[1])
                    if prev is not None:
                        pg, ppc = prev
                        for _ in range(2):
                            if ys:
                                yu = ys.pop()
                                if yu is not None:
                                    y_unit(*yu)
                        for u in range(2):
                            kt = 2 * pg + u
                            nc.tensor.matmul(
                                otCc[:],
                                lhsT=vsb[kt][:, 2 * (DH + 1) : VW],
                                rhs=ppc[u][:],
                                start=(kt == 0), stop=(kt == KT - 1),
                            )
                    prev = (g, pc) if g < KT // 2 else None
                normalize(otCc, otC[qb][:], f"C{qb}")

            ymap = {2: [0], 3: [1, 2]}
            for qb in range(NB):
                units = [
                    (src_qb, qt, pc)
                    for src_qb in ymap.get(qb, [])
                    for qt in range(4)
                    for pc in range(2)
                ]
                attention(qb, list(reversed(units)), c_first=(qb == NB - 1))

            # tail: block 3's y. Head-C matmuls first (otC[3] is ready --
            # phase C ran first), overlapping the A/B normalize; pair
            # matmuls follow in separate PSUM tiles, merged by the add.
            lastq = NB - 1
            cps = {}
            pend = []

            def tail_c(qt, piece):
                lc = otC[lastq][:, qt * P : (qt + 1) * P]
                o0 = piece * 384
                ps = otp.tile([P, 512], F32, tag="ot", name=f"tc{qt}_{piece}")
                nc.tensor.matmul(
                    ps[:, 0:384], lhsT=lc, rhs=wpc_s[0:64, o0 : o0 + 384],
                    start=True, stop=True,
                )
                csb = pexp.tile([P, 384], F32, tag="csb", bufs=4, name=f"cs{qt}_{piece}")
                nc.scalar.copy(csb[:], ps[:, 0:384])
                cps[(qt, piece)] = csb

            def tail_p(qt, piece):
                lp = otP[lastq][:, qt * P : (qt + 1) * P]
                if piece == 0:
                    ysb_tiles[(lastq, qt)] = pexp.tile(
                        [P, C], BF16, tag="y", bufs=4, name=f"ysbL{qt}"
                    )
                ysb = ysb_tiles[(lastq, qt)]
                o0 = piece * 384
                ps = stp.tile([P, 512], F32, tag="st", name=f"tp{qt}_{piece}")
                nc.tensor.matmul(
                    ps[:, 0:384], lhsT=lp, rhs=wpp_s[:, o0 : o0 + 384],
                    start=True, stop=True,
                )
                csb = cps.pop((qt, piece))
                nc.vector.tensor_add(ysb[:, o0 : o0 + 384], ps[:, 0:384], csb[:])
                if piece == 1:
                    nc.sync.dma_start(
                        out[lastq * 512 + qt * P : lastq * 512 + (qt + 1) * P, :],
                        ysb[:],
                    )

            order = [(qt, pc) for qt in range(4) for pc in range(2)]
            # ladder: C-matmuls run ~3 units ahead of pair-matmuls
            emitted_c = 0
            for i, (qt, pc) in enumerate(order):
                while emitted_c < min(len(order), i + 3):
                    tail_c(*order[emitted_c])
                    emitted_c += 1
                tail_p(qt, pc)

    nc.compile()
    return nc


def _get_nc():
    global _nc_cache
    if _nc_cache is None:
        _nc_cache = _build_nc()
    return _nc_cache


def _ktile_major(w):
    # [C, M] -> [128, KJ*M] with contraction tile j at free offset j*M
    M = w.shape[1]
    return np.ascontiguousarray(
        w.reshape(KJ, P, M).transpose(1, 0, 2).reshape(P, KJ * M)
    )


def kernel(x, w_qkv, gate, w_proj, b_proj):
    from concourse import bass_utils

    global LAST_RESULT

    x = np.asarray(x, dtype=np.float32)
    w_qkv = np.asarray(w_qkv, dtype=np.float32)
    gate = np.asarray(gate, dtype=np.float32)
    w_proj = np.asarray(w_proj, dtype=np.float32)
    b_proj = np.asarray(b_proj, dtype=np.float32)

    # ---- host-side layout prep (weights folded/sliced, layout-only for x) ----
    wq_full = w_qkv[:, 0:C] * SCALE
    wk_full = w_qkv[:, C : 2 * C]
    wv_full = w_qkv[:, 2 * C : 3 * C]
    gated_wp = w_proj * np.repeat(gate, DH)[:, None]

    per_hh = []
    for hh in range(4):
        h0 = HL * hh
        cs = slice(h0 * DH, (h0 + HL) * DH)
        wq_np = _ktile_major(wq_full[:, cs]).astype(_BF16)
        wk_np = _ktile_major(wk_full[:, cs]).astype(_BF16)
        wv_pad = np.zeros((C, VW), dtype=np.float32)
        for h in range(HL):
            wv_pad[:, h * (DH + 1) : h * (DH + 1) + DH] = wv_full[
                :, (h0 + h) * DH : (h0 + h + 1) * DH
            ]
        wv_np = _ktile_major(wv_pad).astype(_BF16)
        wp_rows = gated_wp[cs, :]  # [192, 768]
        wpp_np = np.ascontiguousarray(wp_rows[0 : 2 * DH, :]).astype(_BF16)
        wpc_np = np.ascontiguousarray(wp_rows[2 * DH :, :]).astype(_BF16)
        per_hh.append((wq_np, wk_np, wv_np, wpp_np, wpc_np))

    xt_b = []
    for b in range(B):
        xtb = x[b].T.astype(_BF16)  # [C, N]
        xt_b.append(
            [
                _ktile_major(np.ascontiguousarray(xtb[:, n * 512 : (n + 1) * 512]))
                for n in range(NB)
            ]
        )

    in_maps = []
    for c in range(NCORES):
        b, hh = c // 4, c % 4
        wq_np, wk_np, wv_np, wpp_np, wpc_np = per_hh[hh]
        m = {f"xt{n}": xt_b[b][n] for n in range(NB)}
        m.update({"wq": wq_np, "wk": wk_np, "wv": wv_np, "wpp": wpp_np, "wpc": wpc_np})
        in_maps.append(m)

    nc = _get_nc()
    # the first execution of a freshly compiled NEFF occasionally hits a
    # transient NRT_EXEC_UNIT_UNRECOVERABLE; a retry reliably succeeds
    last_exc = None
    for _attempt in range(3):
        try:
            res = bass_utils.run_bass_kernel_spmd(
                nc, in_maps, core_ids=list(range(NCORES)), trace=TRACE
            )
            break
        except Exception as e:  # noqa: BLE001
            last_exc = e
    else:
        raise last_exc
    LAST_RESULT = res

    out = np.empty((B, N, C), dtype=np.float32)
    for b in range(B):
        acc = np.asarray(res.results[4 * b + 0]["out"]).astype(np.float32)
        for hh in range(1, 4):
            acc += np.asarray(res.results[4 * b + hh]["out"]).astype(np.float32)
        out[b] = acc + b_proj[None, :]
    return out
